# revision 24
# baseline (speedup 1.0000x reference)
"""Trainium2 Bass kernel for nn_EntropyModule (GNN message passing, 3-layer
graph conv + neighbor attention + head MLP).

Sharding: points split across 8 cores (2500 each). Each core keeps a full
replicated feature table [20000, 256] bf16 in DRAM (cols 0:128 = x features,
128:131 = point xyz). Neighbor features are fetched with dma_gather
(transpose=True) straight into feature-major SBUF layout. Between layers the
per-core table shard is AllGathered so every core sees the updated table.

All per-edge compute is bf16 (PSUM accumulates f32); residuals/head are f32.
Validated against the reference in numpy: rel err ~2e-3 (see numpy_model.py).
"""
import os
import sys
import numpy as np

sys.path.insert(0, "/opt/trn_rl_repo")

import ml_dtypes

BF16 = ml_dtypes.bfloat16

# problem constants (hardcoded per harness contract)
N, K, D, L = 20000, 16, 128, 3
NCORES = 8
NPC = N // NCORES            # 2500 points per core
PT = 128                     # points per tile
NT = 20                      # tiles per core (2560 padded points)
PTS_PAD = PT * NT            # 2560
TT = PT * K                  # 2048 tokens (edges) per tile
CH = 512                     # matmul free-dim chunk
NCH = TT // CH               # 4 chunks per tile
TBL_C = 256                  # table cols (bf16): 0:128 x | 128:131 pos | pad
BC = 8


def _build(nc_mod, ba_vals):
    """Build the Bass program. Returns (nc, input names meta)."""
    bass = nc_mod["bass"]
    mybir = nc_mod["mybir"]
    tile = nc_mod["tile"]
    masks = nc_mod["masks"]

    f32 = mybir.dt.float32
    bf16 = mybir.dt.bfloat16
    i16 = mybir.dt.int16
    Alu = mybir.AluOpType
    Act = mybir.ActivationFunctionType

    nc = nc_mod["bacc"].Bacc(
        "TRN2", target_bir_lowering=False, debug=False,
        enable_asserts=False, num_devices=NCORES,
        dynamic_dma_scratch_size=int(os.environ.get("KERNEL_DMASCRATCH", 131072)),
    )

    def din(name, shape, dt):
        return nc.dram_tensor(name, shape, dt, kind="ExternalInput").ap()

    def dout(name, shape, dt):
        return nc.dram_tensor(name, shape, dt, kind="ExternalOutput").ap()

    # ---------------- kernel I/O ----------------
    tbl0 = din("tbl0", [N, D], bf16)               # L0 gather table (pos in cols 0:3)
    idxw = din("idxw", [L, 128, PTS_PAD * K // 16], i16)  # wrapped+replicated idx
    inp_fm = din("inp_fm", [3, PTS_PAD], bf16)     # centers, feature-major
    pos_pm = din("pos_pm", [PTS_PAD, 4], bf16)     # centers, point-major (table cols)

    w0a = din("w0a", [3, 3 * 32], bf16)
    w0b = din("w0b", [32, 64], bf16)
    w0c = din("w0c", [64, D], bf16)
    b0a = din("b0a", [32, 1], f32)
    b0b = din("b0b", [64, 1], f32)
    b0c = din("b0c", [D, 1], f32)
    wx1 = din("wx1", [D, D], bf16)
    wg1 = din("wg1", [3, 3 * D], bf16)
    b1 = din("b1", [D, 1], f32)
    wx2 = din("wx2", [D, D], bf16)
    wg2 = din("wg2", [3, 3 * D], bf16)
    b2 = din("b2", [D, 1], f32)
    # attention: col 0 = w1 (per-edge), col 1 = w2 (k=0 anchor)
    wa = din("wa", [L, D, 2], bf16)
    wh1 = din("wh1", [3, D, D], f32)
    bh1 = din("bh1", [D, 1], f32)
    wh2 = din("wh2", [D, D], f32)
    bh2 = din("bh2", [D, 1], f32)
    wh3 = din("wh3", [D, 2 * BC], f32)
    bh3 = din("bh3", [2 * BC, 1], f32)

    mean_o = dout("mean_o", [NPC, BC], f32)
    scale_o = dout("scale_o", [NPC, BC], f32)

    from contextlib import ExitStack
    stk = ExitStack()
    with tile.TileContext(nc) as tc, stk:
        cpool = stk.enter_context(tc.tile_pool(name="const", bufs=1))
        # --------- preload constants ---------
        idx_sb = cpool.tile([128, L * PTS_PAD * K // 16], i16)
        nc.sync.dma_start(out=idx_sb[:], in_=idxw.rearrange("l p s -> p l s"))
        inp_sb = cpool.tile([3, PTS_PAD], bf16)
        nc.sync.dma_start(out=inp_sb[:], in_=inp_fm[:])
        pos_sb = cpool.tile([PT, NT, 4], bf16)
        nc.sync.dma_start(out=pos_sb[:], in_=pos_pm.rearrange("(t p) c -> p t c", p=PT))

        w0a_sb = cpool.tile([3, 3 * 32], bf16)
        nc.sync.dma_start(out=w0a_sb[:], in_=w0a[:])
        w0b_sb = cpool.tile([32, 64], bf16)
        nc.sync.dma_start(out=w0b_sb[:], in_=w0b[:])
        w0c_sb = cpool.tile([64, D], bf16)
        nc.sync.dma_start(out=w0c_sb[:], in_=w0c[:])
        b0a_sb = cpool.tile([32, 1], f32)
        nc.sync.dma_start(out=b0a_sb[:], in_=b0a[:])
        b0b_sb = cpool.tile([64, 1], f32)
        nc.sync.dma_start(out=b0b_sb[:], in_=b0b[:])
        b0c_sb = cpool.tile([D, 1], f32)
        nc.sync.dma_start(out=b0c_sb[:], in_=b0c[:])
        wx1_sb = cpool.tile([D, D], bf16)
        nc.sync.dma_start(out=wx1_sb[:], in_=wx1[:])
        wg1_sb = cpool.tile([3, 3 * D], bf16)
        nc.sync.dma_start(out=wg1_sb[:], in_=wg1[:])
        b1_sb = cpool.tile([D, 1], f32)
        nc.sync.dma_start(out=b1_sb[:], in_=b1[:])
        wx2_sb = cpool.tile([D, D], bf16)
        nc.sync.dma_start(out=wx2_sb[:], in_=wx2[:])
        wg2_sb = cpool.tile([3, 3 * D], bf16)
        nc.sync.dma_start(out=wg2_sb[:], in_=wg2[:])
        b2_sb = cpool.tile([D, 1], f32)
        nc.sync.dma_start(out=b2_sb[:], in_=b2[:])
        wa_sb = cpool.tile([D, L * 2], bf16)
        nc.sync.dma_start(out=wa_sb[:], in_=wa.rearrange("l d c -> d l c"))
        wh1_sb = cpool.tile([D, 3 * D], f32)
        nc.sync.dma_start(out=wh1_sb[:], in_=wh1.rearrange("l a b -> a l b"))
        bh1_sb = cpool.tile([D, 1], f32)
        nc.sync.dma_start(out=bh1_sb[:], in_=bh1[:])
        wh2_sb = cpool.tile([D, D], f32)
        nc.sync.dma_start(out=wh2_sb[:], in_=wh2[:])
        bh2_sb = cpool.tile([D, 1], f32)
        nc.sync.dma_start(out=bh2_sb[:], in_=bh2[:])
        wh3_sb = cpool.tile([D, 2 * BC], f32)
        nc.sync.dma_start(out=wh3_sb[:], in_=wh3[:])
        bh3_sb = cpool.tile([2 * BC, 1], f32)
        nc.sync.dma_start(out=bh3_sb[:], in_=bh3[:])

        id_bf = cpool.tile([128, 128], bf16)
        masks.make_identity(nc, id_bf[:])
        id_f32 = cpool.tile([128, 128], f32)
        masks.make_identity(nc, id_f32[:])
        ones_bf = cpool.tile([1, 128], bf16)
        nc.vector.memset(ones_bf[:], 1.0)

        # persistent residual streams (feature-major, f32)
        x_sb = [cpool.tile([D, PTS_PAD], f32, name=f"x{i}_sb") for i in range(3)]

        # DRAM: AG bounce + tables
        dpool = stk.enter_context(tc.tile_pool(name="dram", bufs=1, space="DRAM"))
        contrib = [dpool.tile([NPC, TBL_C], bf16, name=f"contrib{i}") for i in range(2)]
        tbls = [dpool.tile([N, TBL_C], bf16, addr_space="Shared", name=f"tbl{i + 1}")
                for i in range(2)]

        # --------- per-layer pools ---------
        gpool = stk.enter_context(tc.tile_pool(name="gth", bufs=2))
        hpool = stk.enter_context(tc.tile_pool(name="hfm", bufs=2))
        spool = stk.enter_context(tc.tile_pool(name="small", bufs=2))
        rpool = stk.enter_context(tc.tile_pool(name="rows", bufs=2))
        pp = stk.enter_context(tc.tile_pool(name="pp", bufs=1, space="PSUM"))

        NLAYERS = int(os.environ.get("KERNEL_NLAYERS", L))
        STAGE = int(os.environ.get("KERNEL_STAGE", 5))
        for i in range(NLAYERS, L):
            nc.vector.memset(x_sb[i][:], 0.0)   # debug-only truncated runs
        if STAGE < 4:
            for i in range(NLAYERS):
                nc.vector.memset(x_sb[i][:], 0.0)
        for l in range(NLAYERS):
            gcols = 1 if l == 0 else 2
            gc = 0 if l == 0 else 1          # chunk holding pos rows
            tbl_ap = tbl0[:] if l == 0 else tbls[l - 1][:]
            wxl = (None, wx1_sb, wx2_sb)[l]
            wgl = (None, wg1_sb, wg2_sb)[l]
            bl = (None, b1_sb, b2_sb)[l]

            for t in range(NT):
                rows = min(PT, NPC - t * PT)  # 128, last tile 68
                icol = l * (PTS_PAD * K // 16) + t * (TT // 16)
                gth = gpool.tile([128, gcols, TT], bf16, tag="gth")
                nc.gpsimd.dma_gather(
                    out_ap=gth[:],
                    in_ap=tbl_ap,
                    idxs_ap=idx_sb[:, icol:icol + TT // 16],
                    num_idxs=TT,
                    num_idxs_reg=TT,
                    elem_size=gcols * 128,
                    transpose=True,
                    single_packet=bool(int(os.environ.get("KERNEL_SP", "1"))),
                )
                # geo rows: gth[0:3, gc] = pos ; write sub into 3:6, |sub| into 6:9
                c_b = inp_sb[:, t * PT:(t + 1) * PT].to_broadcast([3, PT, K])
                gsub = spool.tile([3, TT], bf16, tag="gsub")
                gdist = spool.tile([3, TT], bf16, tag="gdist")
                nc.vector.tensor_tensor(
                    out=gsub[:].rearrange("p (n k) -> p n k", k=K),
                    in0=gth[0:3, gc, :].rearrange("p (n k) -> p n k", k=K),
                    in1=c_b, op=Alu.subtract,
                )
                nc.scalar.activation(gdist[:], gsub[:], Act.Abs)

                if STAGE < 2:
                    continue
                h = hpool.tile([D, TT], bf16, tag="h")
                if l == 0:
                    h1 = hpool.tile([32, TT], bf16, tag="h1")
                    h2 = hpool.tile([64, TT], bf16, tag="h2")
                for c in range(NCH):
                    cs = slice(c * CH, (c + 1) * CH)
                    if l == 0:
                        ps1 = pp.tile([32, CH], f32, tag="psh", bufs=3)
                        for gi, g in enumerate((gth[0:3, 0, cs], gsub[:, cs],
                                                gdist[:, cs])):
                            nc.tensor.matmul(ps1[:], w0a_sb[:, gi * 32:(gi + 1) * 32],
                                             g, start=(gi == 0), stop=(gi == 2))
                        nc.scalar.activation(h1[:, cs], ps1[:], Act.Relu, bias=b0a_sb[:, 0:1])
                        ps2 = pp.tile([64, CH], f32, tag="psh", bufs=3)
                        nc.tensor.matmul(ps2[:], w0b_sb[:], h1[:, cs],
                                         start=True, stop=True)
                        nc.scalar.activation(h2[:, cs], ps2[:], Act.Relu, bias=b0b_sb[:, 0:1])
                        ps3 = pp.tile([D, CH], f32, tag="psh", bufs=3)
                        nc.tensor.matmul(ps3[:], w0c_sb[:], h2[:, cs],
                                         start=True, stop=True)
                        nc.scalar.activation(h[:, cs], ps3[:], Act.Relu, bias=b0c_sb[:, 0:1])
                        psh = ps3
                    else:
                        psh = pp.tile([D, CH], f32, tag="psh", bufs=3)
                        nc.tensor.matmul(psh[:], wxl[:], gth[0:D, 0, cs],
                                         start=True, stop=False)
                        for gi, g in enumerate((gth[0:3, 1, cs], gsub[:, cs],
                                                gdist[:, cs])):
                            nc.tensor.matmul(psh[:], wgl[:, gi * D:(gi + 1) * D],
                                             g, start=False, stop=(gi == 2))
                        nc.scalar.activation(h[:, cs], psh[:], Act.Relu, bias=bl[:, 0:1])

                # ---- attention scores: u + v in psum row 0
                if STAGE < 3:
                    continue
                s_lr = spool.tile([1, TT], f32, tag="slr")
                for c in range(NCH):
                    cs = slice(c * CH, (c + 1) * CH)
                    pss = pp.tile([1, CH], f32, tag="pss", bufs=2)
                    nc.tensor.matmul(pss[:], wa_sb[:, 2 * l:2 * l + 1], h[:, cs],
                                     start=True, stop=False)
                    h_sel = h[:, c * CH:(c + 1) * CH:K].to_broadcast([D, CH // K, K])
                    nc.tensor.matmul(pss[:].rearrange("p (n k) -> p n k", k=K),
                                     wa_sb[:, 2 * l + 1:2 * l + 2], h_sel,
                                     start=False, stop=True)
                    nc.scalar.activation(s_lr[:, cs], pss[:], Act.Copy)
                # fold to [point, k] layout, softmax (no max-sub), back to row
                s2d = spool.tile([PT, K], f32, tag="s2d")
                nc.sync.dma_start(out=s2d[:],
                                  in_=s_lr[:].rearrange("o (p k) -> o p k", p=PT))
                # s = leaky_relu(s2d + ba, 0.2); e = exp(s)
                sb_ = spool.tile([PT, K], f32, tag="sb_")
                nc.vector.tensor_scalar_add(sb_[:], s2d[:], ba_vals[l])
                sc_ = spool.tile([PT, K], f32, tag="sc_")
                nc.vector.tensor_scalar_mul(sc_[:], sb_[:], 0.2)
                nc.vector.tensor_tensor(out=sb_[:], in0=sb_[:], in1=sc_[:], op=Alu.max)
                e2d = spool.tile([PT, K], f32, tag="e2d")
                nc.scalar.activation(e2d[:], sb_[:], Act.Exp)
                z2d = spool.tile([PT, 1], f32, tag="z2d")
                nc.vector.reduce_sum(z2d[:], e2d[:], axis=mybir.AxisListType.X)
                r2d = spool.tile([PT, 1], f32, tag="r2d")
                nc.vector.reciprocal(r2d[:], z2d[:])
                w2db = spool.tile([PT, K], bf16, tag="w2db")
                nc.vector.tensor_scalar(out=w2db[:], in0=e2d[:], scalar1=r2d[:, 0:1],
                                        scalar2=None, op0=Alu.mult)
                if STAGE < 4:
                    continue
                w_row = spool.tile([1, TT], bf16, tag="wrow")
                nc.sync.dma_start(out=w_row[:].rearrange("o (p k) -> o p k", p=PT),
                                  in_=w2db[:])
                # weighted sum over k: replicate w across partitions via PE,
                # multiply, segment-reduce
                wh = hpool.tile([D, TT], bf16, tag="wh")
                for c in range(NCH):
                    cs = slice(c * CH, (c + 1) * CH)
                    psw = pp.tile([D, CH], f32, tag="psw", bufs=2)
                    nc.tensor.matmul(psw[:], ones_bf[:], w_row[:, cs],
                                     start=True, stop=True)
                    nc.vector.tensor_tensor(out=wh[:, cs], in0=h[:, cs], in1=psw[:],
                                            op=Alu.mult)
                o_sb = spool.tile([D, PT], f32, tag="osb")
                nc.vector.reduce_sum(o_sb[:], wh.rearrange("p (n k) -> p n k", k=K),
                                     axis=mybir.AxisListType.X)
                # elu + residual -> x_l feature-major f32
                mn = spool.tile([D, PT], f32, tag="mn")
                nc.vector.tensor_scalar_min(mn[:], o_sb[:], 0.0)
                ex = spool.tile([D, PT], f32, tag="ex")
                nc.scalar.activation(ex[:], mn[:], Act.Exp)
                rl = spool.tile([D, PT], f32, tag="rl")
                nc.vector.tensor_scalar(out=rl[:], in0=o_sb[:], scalar1=0.0,
                                        scalar2=-1.0, op0=Alu.max, op1=Alu.add)
                tcols = slice(t * PT, (t + 1) * PT)
                if l == 0:
                    nc.vector.tensor_tensor(out=x_sb[0][:, tcols], in0=rl[:],
                                            in1=ex[:], op=Alu.add)
                else:
                    el = spool.tile([D, PT], f32, tag="el")
                    nc.vector.tensor_tensor(out=el[:], in0=rl[:], in1=ex[:], op=Alu.add)
                    nc.vector.tensor_tensor(out=x_sb[l][:, tcols], in0=el[:],
                                            in1=x_sb[l - 1][:, tcols], op=Alu.add)
                # table row write (layers 0,1): bf16 cast -> transpose -> rows
                if l < 2 and STAGE >= 5:
                    xb = spool.tile([D, PT], bf16, tag="xb")
                    nc.vector.tensor_copy(out=xb[:], in_=x_sb[l][:, tcols])
                    pst = pp.tile([PT, D], bf16, tag="pst", bufs=1)
                    nc.tensor.transpose(pst[:], xb[:], id_bf[:])
                    row = rpool.tile([PT, TBL_C], bf16, tag="row")
                    nc.scalar.activation(row[:, 0:D], pst[:], Act.Copy)
                    nc.vector.tensor_copy(out=row[:, D:D + 4], in_=pos_sb[:, t, :])
                    nc.vector.memset(row[:, D + 4:], 0.0)
                    nc.sync.dma_start(out=contrib[l][t * PT:t * PT + rows, :],
                                      in_=row[:rows, :])

            if l < 2:
                if os.environ.get("KERNEL_NO_COLLECTIVE"):
                    # hang-debug mode: local copy instead of AllGather
                    nc.sync.dma_start(out=tbls[l][0:NPC, :], in_=contrib[l][:])
                else:
                    nc.gpsimd.collective_compute(
                        "AllGather", mybir.AluOpType.bypass,
                        replica_groups=[list(range(NCORES))],
                        ins=[contrib[l][:]], outs=[tbls[l][:]],
                    )

        # ---------------- head MLP (f32, feature-major) ----------------
        f1 = cpool.tile([D, PTS_PAD], f32)
        f2 = cpool.tile([D, PTS_PAD], f32)
        ms = cpool.tile([2 * BC, PTS_PAD], f32)
        for c in range(PTS_PAD // CH):
            cs = slice(c * CH, (c + 1) * CH)
            psf = pp.tile([D, CH], f32, tag="psh", bufs=3)
            for i in range(3):
                nc.tensor.matmul(psf[:], wh1_sb[:, i * D:(i + 1) * D], x_sb[i][:, cs],
                                 start=(i == 0), stop=(i == 2))
            nc.scalar.activation(f1[:, cs], psf[:], Act.Relu, bias=bh1_sb[:, 0:1])
            psg = pp.tile([D, CH], f32, tag="psh", bufs=3)
            nc.tensor.matmul(psg[:], wh2_sb[:], f1[:, cs], start=True, stop=True)
            nc.scalar.activation(f2[:, cs], psg[:], Act.Relu, bias=bh2_sb[:, 0:1])
            psm = pp.tile([2 * BC, CH], f32, tag="pss", bufs=2)
            nc.tensor.matmul(psm[:], wh3_sb[:], f2[:, cs], start=True, stop=True)
            nc.scalar.activation(ms[:, cs], psm[:], Act.Identity, bias=bh3_sb[:, 0:1])
        # transpose to point-major, exp the scale half, DMA out
        for t in range(NT):
            rows = min(PT, NPC - t * PT)
            tcols = slice(t * PT, (t + 1) * PT)
            pso = pp.tile([PT, 2 * BC], f32, tag="pst", bufs=1)
            nc.tensor.transpose(pso[:], ms[:, tcols], id_f32[0:2 * BC, 0:2 * BC])
            mo = spool.tile([PT, BC], f32, tag="mo")
            nc.vector.tensor_copy(out=mo[:], in_=pso[:, 0:BC])
            so = spool.tile([PT, BC], f32, tag="so")
            nc.scalar.activation(so[:], pso[:, BC:2 * BC], Act.Exp)
            nc.sync.dma_start(out=mean_o[t * PT:t * PT + rows, :], in_=mo[:rows, :])
            nc.sync.dma_start(out=scale_o[t * PT:t * PT + rows, :], in_=so[:rows, :])

    nc.compile()
    return nc


_CACHE = {}


def _get_nc(ba_vals=(0.0, 0.0, 0.0)):
    if "nc" not in _CACHE:
        import concourse.bass as bass
        import concourse.mybir as mybir
        import concourse.tile as tile
        import concourse.bacc as bacc
        import concourse.masks as masks
        _CACHE["nc"] = _build(dict(bass=bass, mybir=mybir, tile=tile,
                                   bacc=bacc, masks=masks), [float(b) for b in ba_vals])
    return _CACHE["nc"]


def _prep_inputs(input, knn_idx, mlp_params, attn_params, out_params):
    """Host-side sharding/layout prep. Returns list of 8 in_maps."""
    inp = np.asarray(input, np.float32)[0]            # [N, 3]
    knn = np.asarray(knn_idx)[:, 0]                   # [L, N, K]

    def A(x):
        return np.asarray(x, np.float32)

    mlp = [[(A(W), A(b)) for W, b in layer] for layer in mlp_params]
    attn = [(A(W), A(b)) for W, b in attn_params]
    outp = [(A(W), A(b)) for W, b in out_params]

    # global tables / weights (shared by all cores)
    tbl0 = np.zeros([N, D], BF16)
    tbl0[:, 0:3] = inp.astype(BF16)

    W1, b1v = mlp[0][0]
    # geo weight pack: [3, 3*out] = [pos | sub | dist] along free dim
    W1f = np.concatenate([W1[0:3] + W1[3:6], W1[6:9], W1[9:12]], axis=1)

    def gpack(W):  # [9, out] -> [3, 3*out]
        return np.concatenate([W[0:3], W[3:6], W[6:9]], axis=1)

    com = dict(
        tbl0=tbl0,
        w0a=W1f.astype(BF16), b0a=b1v.reshape(-1, 1),
        w0b=mlp[0][1][0].astype(BF16), b0b=mlp[0][1][1].reshape(-1, 1),
        w0c=mlp[0][2][0].astype(BF16), b0c=mlp[0][2][1].reshape(-1, 1),
        wx1=mlp[1][0][0][0:D].astype(BF16), wg1=gpack(mlp[1][0][0][D:]).astype(BF16),
        b1=mlp[1][0][1].reshape(-1, 1),
        wx2=mlp[2][0][0][0:D].astype(BF16), wg2=gpack(mlp[2][0][0][D:]).astype(BF16),
        b2=mlp[2][0][1].reshape(-1, 1),
        wa=np.stack([np.stack([attn[l][0][0:D, 0], attn[l][0][D:, 0]], axis=1)
                     for l in range(L)]).astype(BF16),
        wh1=np.stack([outp[0][0][i * D:(i + 1) * D] for i in range(3)]),
        bh1=outp[0][1].reshape(-1, 1),
        wh2=outp[1][0], bh2=outp[1][1].reshape(-1, 1),
        wh3=outp[2][0], bh3=outp[2][1].reshape(-1, 1),
    )

    in_maps = []
    for c in range(NCORES):
        r0 = c * NPC
        pts = inp[r0:r0 + NPC]                         # [2500, 3]
        pts_pad = np.zeros([PTS_PAD, 3], np.float32)
        pts_pad[:NPC] = pts
        # idx per layer, point-major tokens, wrapped [128, TT/16] replicated
        idxw = np.zeros([L, 128, PTS_PAD * K // 16], np.int16)
        for l in range(L):
            flat = np.zeros([PTS_PAD * K], np.int16)
            flat[:NPC * K] = knn[l, r0:r0 + NPC].reshape(-1).astype(np.int16)
            wrap = flat.reshape(-1, 16).T              # [16, PTS_PAD*K/16]
            idxw[l] = np.tile(wrap, (8, 1))
        pos_pm = np.zeros([PTS_PAD, 4], BF16)
        pos_pm[:NPC, 0:3] = pts.astype(BF16)
        m = dict(com)
        m.update(
            idxw=idxw,
            inp_fm=np.ascontiguousarray(pts_pad.T).astype(BF16),
            pos_pm=pos_pm,
        )
        in_maps.append(m)
    return in_maps


def kernel(input, knn_idx, mlp_params, attn_params, out_params):
    from concourse import bass_utils
    nc = _get_nc([float(np.asarray(attn_params[l][1]).reshape(-1)[0])
                  for l in range(L)])
    in_maps = _prep_inputs(input, knn_idx, mlp_params, attn_params, out_params)
    res = bass_utils.run_bass_kernel_spmd(
        nc, in_maps, core_ids=list(range(NCORES)),
        trace=bool(int(os.environ.get("KERNEL_TRACE", "0"))),
    )
    _CACHE["last_results"] = res
    mean = np.concatenate([res.results[c]["mean_o"] for c in range(NCORES)], axis=0)
    scale = np.concatenate([res.results[c]["scale_o"] for c in range(NCORES)], axis=0)
    return mean[None], scale[None]
